# revision 51
# baseline (speedup 1.0000x reference)
"""GCN (3-layer + mean-pool + linear head) on 8 TRN2 NeuronCores.

Strategy (dst-sharded message passing, v2):
  - Nodes split into 8 slices of 12500; core i owns dst slice i and the edges
    pointing into it.  norm = dis[src]*dis[dst] folded into per-node scalings.
  - L1 aggregation needs no device gather at all: the host packs a per-dst
    padded stream xe[p, b, f, j] = dis[src]*x[src] (fp16) and the device does
    one strided vector reduce per 128-dst block.  (S x) W1 == S (x W1).
  - Per block: z -> W1 GEMM -> relu -> transpose -> W2 GEMM -> m2 (fp16).
    m2 kept in SBUF (m2own) and written to DRAM for the AllGather.
  - One AllGather publishes the fp16 message plane (256B rows).
  - L2 aggregation: swdge dma_gather of fp16 256B message rows (4 queues,
    per-(block,chunk) calls, trailing negative indices skip padding slots,
    partial-K matmul on each bucket's last tile), then one-hot fp8
    "segment matrix" matmuls accumulate 128 edges/instr into PSUM per block.
    Self-loop messages are added from SBUF (no gather slots for them).
  - W3+pool fold: q = sum_b h2_b^T @ C'_b accumulates in one PSUM bank
    across all blocks; W3 is applied once after the AllReduce.
  - L3 + mean-pool collapse into pooled = C' @ (h2 W3) with C' precomputed
    on host from indices/degrees only; one AllReduce of [128, 64] partials.
"""
import os
import sys

sys.path.insert(0, "/opt/trn_rl_repo")

import numpy as np
import ml_dtypes

N = 100000
E = 1600000
F = 3
H = 128
C = 4
G = 64
P = 8
NP = N // P            # 12500 nodes per core
BLK = 128
NBLK = (NP + BLK - 1) // BLK   # 98 (last block has 84 rows)
BPG = 4                        # dst blocks per psum-bank group
NGRP = (NBLK + BPG - 1) // BPG # 25
NCHUNK = 4                     # src chunks for swdge gather (int16 rel idx)
CHUNK = N // NCHUNK            # 25000 (indirect-mode layout only)
# quarter regions of each core's slice; t_mfull is quarter-major so the
# AllGather can be split into 4 partials issued in 2 pairs (2 waves)
QROW = [0, 3136, 6272, 9408, NP]
QSIZE = [QROW[q + 1] - QROW[q] for q in range(4)]
QBASE = np.cumsum([0] + [P * s for s in QSIZE]).tolist()

_CACHE = {}


def _host_prep(x, edge_index, batch):
    f8 = ml_dtypes.float8_e4m3
    x = np.asarray(x, np.float32)
    loops = np.arange(N, dtype=np.int64)
    src = np.concatenate([np.asarray(edge_index[0], np.int64), loops])
    dst = np.concatenate([np.asarray(edge_index[1], np.int64), loops])
    deg = np.bincount(dst, minlength=N).astype(np.float32)
    dis = np.where(deg > 0, deg ** np.float32(-0.5), np.float32(0)).astype(np.float32)

    core = dst // NP
    ld = dst % NP
    blk = ld // BLK
    dpos = ld % BLK

    # ---- L1: per-dst padded xe stream -----------------------------------
    # Dmax per block, equalized across cores (same compiled program).
    degpad = np.zeros(P * NBLK * BLK, np.float32)
    for i in range(P):
        degpad[i * NBLK * BLK: i * NBLK * BLK + NP] = deg[i * NP:(i + 1) * NP]
    Dmax = degpad.reshape(P, NBLK, BLK).max(axis=(0, 2)).astype(np.int64)  # [NBLK]
    xoff = np.zeros(NBLK + 1, np.int64)
    xoff[1:] = np.cumsum(3 * Dmax)
    C1 = int(xoff[-1])

    # rank of each edge within its dst node's list
    order = np.argsort(dst, kind="stable")
    starts = np.zeros(N + 1, np.int64)
    starts[1:] = np.cumsum(np.bincount(dst, minlength=N))
    rank = np.empty(len(dst), np.int64)
    rank[order] = np.arange(len(dst)) - starts[dst[order]]

    vals = (dis[src][:, None] * x[src]).astype(np.float16)  # [Etot, 3]

    xe_cores = []
    for i in range(P):
        sel = np.where(core == i)[0]
        xe = np.zeros((BLK, C1), np.float16)
        bb, pp, rr = blk[sel], dpos[sel], rank[sel]
        base = xoff[bb] + rr
        for f in range(F):
            xe[pp, base + f * Dmax[bb]] = vals[sel, f]
        xe_cores.append(xe)

    # ---- L2: slot layout (real edges only, no self-loops) ---------------
    gather_mode = os.environ.get("GCN_GATHER", "swdge")
    src2 = src[:E]
    dst2 = dst[:E]
    core2 = dst2 // NP
    ld2 = dst2 % NP
    blk2 = ld2 // BLK
    dloc2 = ld2 % BLK

    if gather_mode == "indirect":
        key2 = core2 * NBLK + blk2
        cnt = np.bincount(key2, minlength=P * NBLK).reshape(P, NBLK)
        T2 = np.maximum(1, -(-cnt.max(axis=0) // BLK)).astype(np.int64)  # [NBLK]
        toff2 = np.zeros(NBLK + 1, np.int64)
        toff2[1:] = np.cumsum(T2)
        TOT2 = int(toff2[-1])

        off_cores = []
        seg_cores = []
        for i in range(P):
            sel = np.where(core2 == i)[0]
            bb = blk2[sel]
            # sort by (block, src) for HBM locality within buckets
            o2 = np.lexsort((src2[sel], bb))
            sel = sel[o2]
            bb = blk2[sel]
            bc = np.bincount(bb, minlength=NBLK)
            bstart = np.zeros(NBLK, np.int64)
            bstart[1:] = np.cumsum(bc)[:-1]
            rnk = np.arange(len(sel)) - bstart[bb]
            slot = toff2[bb] * BLK + rnk
            tt = slot // BLK
            ppos = slot % BLK
            offs = np.zeros((BLK, TOT2), np.int32)
            offs[ppos, tt] = src2[sel].astype(np.int32)
            seg = np.zeros((BLK, TOT2 * BLK), f8)
            seg[ppos, tt * BLK + dloc2[sel]] = np.float32(1.0).astype(f8)
            off_cores.append(offs)
            seg_cores.append(np.asarray(seg))
        l2 = dict(mode="indirect", T2=T2, toff2=toff2, TOT2=TOT2)
    else:
        # swdge quarter layout: t_mfull quarter-major; bucket per
        # (block, src-quarter); tiles laid out wave-major (chunks {0,1}
        # then {2,3}) so wave-0 gathers only need the first two partial
        # AllGathers, which fire after half of phase 1.
        lsrc = src2 % NP
        csrc = src2 // NP
        qsz = np.asarray(QSIZE)
        qrw = np.asarray(QROW)
        chunk2 = np.searchsorted(qrw[1:4], lsrc, side="right")
        srel2 = (csrc * qsz[chunk2] + (lsrc - qrw[chunk2])).astype(np.int16)
        key2 = (core2 * NBLK + blk2) * NCHUNK + chunk2
        cnt = np.bincount(key2, minlength=P * NBLK * NCHUNK).reshape(P, NBLK, NCHUNK)
        T2c = np.maximum(1, -(-cnt.max(axis=0) // BLK)).astype(np.int64)  # [NBLK, NCHUNK]
        tile_off = np.zeros((NBLK, NCHUNK), np.int64)
        gc_base = np.zeros((NGRP, NCHUNK), np.int64)
        gc_ntiles = np.zeros((NGRP, NCHUNK), np.int64)
        tt_ = 0
        for w in range(2):
            for g in range(NGRP):
                for c in (2 * w, 2 * w + 1):
                    gc_base[g, c] = tt_
                    for b in range(g * BPG, min((g + 1) * BPG, NBLK)):
                        tile_off[b, c] = tt_
                        tt_ += T2c[b, c]
                    gc_ntiles[g, c] = tt_ - gc_base[g, c]
        TOT2 = int(tt_)

        # valid slot count per bucket (SPMD-constant): exact max over cores
        Vbc = cnt.max(axis=0)                          # [NBLK, NCHUNK]
        Vbc = np.maximum(np.minimum(Vbc, T2c * BLK), 16).astype(np.int64)

        off_cores = []
        seg_cores = []
        for i in range(P):
            sel = np.where(core2 == i)[0]
            bb, cc2, dd, ss = blk2[sel], chunk2[sel], dloc2[sel], srel2[sel]
            bucket = bb * NCHUNK + cc2
            o2 = np.lexsort((ss, bucket))
            bb, cc2, dd, ss, bucket = bb[o2], cc2[o2], dd[o2], ss[o2], bucket[o2]
            bc = np.bincount(bucket, minlength=NBLK * NCHUNK)
            bstart = np.zeros(NBLK * NCHUNK, np.int64)
            bstart[1:] = np.cumsum(bc)[:-1]
            rnk = np.arange(len(sel)) - bstart[bucket]
            slot = tile_off.reshape(-1)[bucket] * BLK + rnk
            # -1 beyond the valid count (descriptors skipped), 0-dummies
            # between this core's count and the equalized valid count.
            idx16 = np.full(TOT2 * BLK, -1, np.int16)
            for b in range(NBLK):
                for c in range(NCHUNK):
                    s0 = tile_off[b, c] * BLK
                    idx16[s0:s0 + Vbc[b, c]] = 0
            idx16[slot] = ss
            idxw = np.zeros((BLK, TOT2 * 8), np.int16)
            wr = idx16.reshape(-1, 16).T  # [16, TOT2*8]
            for r in range(8):
                idxw[16 * r:16 * r + 16, :] = wr
            seg = np.zeros((BLK, TOT2 * BLK), f8)
            seg[slot % BLK, (slot // BLK) * BLK + dd] = np.float32(1.0).astype(f8)
            off_cores.append(idxw)
            seg_cores.append(np.asarray(seg))
        l2 = dict(mode="swdge", T2c=T2c, tile_off=tile_off, gc_base=gc_base,
                  gc_ntiles=gc_ntiles, TOT2=TOT2, Vbc=Vbc,
                  TGC_MAX=int(gc_ntiles.max()))

    # ---- per-node dst scalings + pooled C' matrix -----------------------
    batch = np.asarray(batch, np.int64)
    cntg = np.bincount(batch, minlength=G).astype(np.float32)
    cmat = np.zeros((G, N), np.float32)
    np.add.at(cmat, (batch[dst], src), (dis[src] * dis[dst]).astype(np.float32))
    cmat /= np.maximum(cntg, 1.0)[:, None]

    dso_cores = []
    cp_cores = []
    for i in range(P):
        dso = np.zeros(NBLK * BLK, np.float32)
        dso[:NP] = dis[i * NP:(i + 1) * NP]
        dso = dso.reshape(NBLK, BLK).T.copy()  # [128, NBLK]
        cpc = np.zeros((NBLK * BLK, G), np.float32)
        cpc[:NP, :] = cmat[:, i * NP:(i + 1) * NP].T
        cp = cpc.reshape(NBLK, BLK, G).transpose(1, 0, 2).reshape(BLK, NBLK * G).copy()
        dso_cores.append(dso)
        cp_cores.append(cp)

    meta = dict(Dmax=Dmax, xoff=xoff, C1=C1, l2=l2)
    per_core = [dict(xe=xe_cores[i], offs=off_cores[i], seg=seg_cores[i],
                     dso=dso_cores[i], cp=cp_cores[i]) for i in range(P)]
    return meta, per_core


def _build(meta):
    import concourse.bacc as bacc
    import concourse.mybir as mybir
    import concourse.tile as tile
    from concourse.bass import IndirectOffsetOnAxis

    dt = mybir.dt
    AF = mybir.ActivationFunctionType
    ALU = mybir.AluOpType

    Dmax = meta["Dmax"]
    xoff = meta["xoff"]
    C1 = meta["C1"]
    l2 = meta["l2"]
    mode = l2["mode"]
    TOT2 = l2["TOT2"]

    nc = bacc.Bacc("TRN2", target_bir_lowering=False, debug=False,
                   num_devices=P, num_swdge_queues=4)

    # ---- dram tensors ----
    t_xe = nc.dram_tensor("xe", [BLK, C1], dt.float16, kind="ExternalInput").ap()
    if mode == "indirect":
        t_off = nc.dram_tensor("offs", [BLK, TOT2], dt.int32,
                               kind="ExternalInput").ap()
    else:
        t_off = nc.dram_tensor("offs", [BLK, TOT2 * 8], dt.int16,
                               kind="ExternalInput").ap()
    t_seg = nc.dram_tensor("seg", [BLK, TOT2 * BLK], dt.float8e4,
                           kind="ExternalInput").ap()
    t_dso = nc.dram_tensor("dso", [BLK, NBLK], dt.float32, kind="ExternalInput").ap()
    t_cp = nc.dram_tensor("cp", [BLK, NBLK * G], dt.float32, kind="ExternalInput").ap()
    t_w1 = nc.dram_tensor("w1", [F, H], dt.float32, kind="ExternalInput").ap()
    t_w2 = nc.dram_tensor("w2", [H, H], dt.float32, kind="ExternalInput").ap()
    t_w3 = nc.dram_tensor("w3", [H, H], dt.float32, kind="ExternalInput").ap()
    t_wl = nc.dram_tensor("wl", [H, C], dt.float32, kind="ExternalInput").ap()
    t_b1 = nc.dram_tensor("b1b", [BLK, 1], dt.float32, kind="ExternalInput").ap()
    t_b2 = nc.dram_tensor("b2b", [BLK, H], dt.float32, kind="ExternalInput").ap()
    t_b3 = nc.dram_tensor("b3c", [BLK, 1], dt.float32, kind="ExternalInput").ap()
    t_bl = nc.dram_tensor("blc", [C, 1], dt.float32, kind="ExternalInput").ap()
    t_id = nc.dram_tensor("ident", [BLK, BLK], dt.float32, kind="ExternalInput").ap()
    t_out = nc.dram_tensor("out", [C, G], dt.float32, kind="ExternalOutput").ap()

    t_min = nc.dram_tensor("m_in", [NP, H], dt.float16, kind="Internal").ap()
    t_mfull = nc.dram_tensor("m_full", [N, H], dt.float16, kind="Internal",
                             addr_space="Shared").ap()
    t_arin = nc.dram_tensor("arin", [BLK, G], dt.float32, kind="Internal").ap()
    t_arout = nc.dram_tensor("arout", [BLK, G], dt.float32, kind="Internal",
                             addr_space="Shared").ap()
    debug = os.environ.get("GCN_DEBUG", "0") == "1"
    t_dh1 = t_dh2 = None
    if debug:
        t_dh1 = nc.dram_tensor("dbg_h1", [NP, H], dt.float32, kind="ExternalOutput").ap()
        t_dh2 = nc.dram_tensor("dbg_h2", [NP, H], dt.float32, kind="ExternalOutput").ap()

    with tile.TileContext(nc) as tc:
        with tc.tile_pool(name="const", bufs=1) as cpool:
            w1s = cpool.tile([F, H], dt.float32)
            w2s = cpool.tile([H, H], dt.float32)
            w3s = cpool.tile([H, H], dt.float32)
            wls = cpool.tile([H, C], dt.float32)
            b1s = cpool.tile([BLK, 1], dt.float32)
            b2s = cpool.tile([BLK, H], dt.float32)
            b3s = cpool.tile([BLK, 1], dt.float32)
            bls = cpool.tile([C, 1], dt.float32)
            dsos = cpool.tile([BLK, NBLK], dt.float32)
            cps = cpool.tile([BLK, NBLK * G], dt.float32)
            ids = cpool.tile([BLK, BLK], dt.float32)
            m2o = cpool.tile([BLK, NBLK * H], dt.float32)
            acc = cpool.tile([BLK, NBLK * H], dt.float32)
            # head strips: preload the first NHEAD groups' wave-0 idx/seg
            # so gathers can start as soon as the first AllGather pair lands
            # (otherwise these loads queue behind all phase-1 DMA issue)
            NHEAD = 3
            head_it = {}
            head_st = {}
            if mode != "indirect":
                gcn = l2["gc_ntiles"]
                gcb = l2["gc_base"]
                for g_ in range(NHEAD):
                    for c_ in (0, 1):
                        nt_ = int(gcn[g_, c_])
                        head_it[(g_, c_)] = cpool.tile(
                            [BLK, nt_ * 8], dt.int16, name=f"hit{g_}_{c_}")
                        head_st[(g_, c_)] = cpool.tile(
                            [BLK, nt_ * BLK], dt.float8e4, name=f"hst{g_}_{c_}")
            loads = [(w1s, t_w1), (w2s, t_w2), (w3s, t_w3), (wls, t_wl),
                     (b1s, t_b1), (b2s, t_b2), (b3s, t_b3), (bls, t_bl),
                     (dsos, t_dso), (cps, t_cp), (ids, t_id)]
            offs = None
            if mode == "indirect":
                offs = cpool.tile([BLK, TOT2], dt.int32)
                loads.append((offs, t_off))
            for dst_t, src_t in loads:
                nc.sync.dma_start(dst_t[:], src_t[:])

            # ---- phase 1: L1 reduce + W1 + W2 per block ----
            with tc.tile_pool(name="xep", bufs=1) as xep, \
                 tc.tile_pool(name="p1", bufs=6) as p1, \
                 tc.tile_pool(name="ps_zt", bufs=2, space="PSUM") as pzt, \
                 tc.tile_pool(name="ps_h1", bufs=2, space="PSUM") as pph, \
                 tc.tile_pool(name="ps_m2", bufs=2, space="PSUM") as ppm:
                xe_sb = xep.tile([BLK, C1], dt.float16)
                half = C1 // 2
                nc.sync.dma_start(xe_sb[:, 0:half], t_xe[:, 0:half])
                nc.sync.dma_start(xe_sb[:, half:C1], t_xe[:, half:C1])
                for (g_, c_), it_ in head_it.items():
                    nt_ = int(l2["gc_ntiles"][g_, c_])
                    base_ = int(l2["gc_base"][g_, c_])
                    nc.sync.dma_start(it_[:],
                                      t_off[:, base_ * 8:(base_ + nt_) * 8])
                    nc.scalar.dma_start(
                        head_st[(g_, c_)][:],
                        t_seg[:, base_ * BLK:(base_ + nt_) * BLK])
                for b in range(NBLK):
                    D = int(Dmax[b])
                    a0 = int(xoff[b])
                    z = p1.tile([BLK, F], dt.float32, tag="z", name=f"z{b}")
                    nc.vector.tensor_reduce(
                        z[:],
                        xe_sb[:, a0:a0 + 3 * D].rearrange("p (f d) -> p f d", d=D),
                        axis=mybir.AxisListType.X, op=ALU.add)
                    # fold dis_dst before W1: h1 = relu((dis*z) W1 + b1)
                    zs = p1.tile([BLK, F], dt.float32, tag="zs", name=f"zs{b}")
                    nc.vector.tensor_scalar(zs[:], z[:], dsos[:, b:b + 1], None,
                                            op0=ALU.mult)
                    zt = pzt.tile([F, BLK], dt.float32, tag="zt", name=f"zt{b}")
                    nc.tensor.transpose(zt[:], zs[:], ids[:])
                    zts = p1.tile([F, BLK], dt.float32, tag="zts", name=f"zts{b}")
                    nc.vector.tensor_copy(zts[:], zt[:])
                    # h1T [fout, dst] = relu(W1^T zT + b1)
                    ph = pph.tile([BLK, BLK], dt.float32, tag="ph", name=f"ph{b}")
                    nc.tensor.matmul(ph[:], lhsT=w1s[:], rhs=zts[:],
                                     start=True, stop=True)
                    h1t = p1.tile([BLK, BLK], dt.float32, tag="h1t", name=f"h1t{b}")
                    nc.scalar.activation(h1t[:], ph[:], AF.Relu,
                                         bias=b1s[:, 0:1])
                    # m2 [node, fout] = dis * (h1 W2) = dis * (h1T^T W2)
                    pm = ppm.tile([BLK, H], dt.float32, tag="pm", name=f"pm{b}")
                    nc.tensor.matmul(pm[:], lhsT=h1t[:], rhs=w2s[:],
                                     start=True, stop=True)
                    nc.scalar.activation(m2o[:, b * H:(b + 1) * H], pm[:], AF.Copy,
                                         scale=dsos[:, b:b + 1])
                    mh = p1.tile([BLK, H], dt.float16, tag="mh", name=f"mh{b}")
                    nc.vector.tensor_copy(mh[:], m2o[:, b * H:(b + 1) * H])
                    rb = min(BLK, NP - b * BLK)
                    nc.sync.dma_start(t_min[b * BLK:b * BLK + rb, :], mh[0:rb, :])
                    if debug:
                        nc.sync.dma_start(t_dh1[b * BLK:b * BLK + rb, :],
                                          h1t[0:rb, :])

            # ---- AllGather fp16 message plane ----
            t_msrc = t_mfull
            if mode == "indirect":
                nc.gpsimd.collective_compute(
                    "AllGather", mybir.AluOpType.bypass,
                    replica_groups=[list(range(P))],
                    ins=[t_min[:]], outs=[t_mfull[:]])

            # ---- phase 2: gather + one-hot matmul agg + L3 fold ----
            qctr = [0]

            def nextq():
                q = qctr[0] % 4
                qctr[0] += 1
                return q

            with tc.tile_pool(name="g2", bufs=12) as g2p, \
                 tc.tile_pool(name="seg2", bufs=8) as seg2p, \
                 tc.tile_pool(name="idx2", bufs=8) as idx2p, \
                 tc.tile_pool(name="blk2", bufs=4) as blk2p, \
                 tc.tile_pool(name="ps_agg", bufs=7, space="PSUM") as psagg, \
                 tc.tile_pool(name="ps_e", bufs=1, space="PSUM") as pse:
                # q[f, g] = sum_b h2_b^T @ cp_b accumulates across all blocks
                qreg = pse.tile([BLK, G], dt.float32)

                def epilogue(b, reg, with_acc=(mode != "indirect")):
                    h2 = blk2p.tile([BLK, H], dt.float32, tag="h2", name=f"h2{b}")
                    if with_acc:
                        # reg holds the wave-1 partial; acc holds wave 0
                        nc.vector.tensor_tensor(h2[:], reg,
                                                acc[:, b * H:(b + 1) * H],
                                                op=ALU.add)
                        nc.vector.tensor_tensor(h2[:], h2[:],
                                                m2o[:, b * H:(b + 1) * H],
                                                op=ALU.add)
                    else:
                        nc.vector.tensor_tensor(h2[:], reg,
                                                m2o[:, b * H:(b + 1) * H],
                                                op=ALU.add)
                    nc.scalar.activation(h2[:], h2[:], AF.Copy,
                                         scale=dsos[:, b:b + 1])
                    nc.vector.tensor_tensor(h2[:], h2[:], b2s[:], op=ALU.add)
                    nc.scalar.activation(h2[:], h2[:], AF.Relu)
                    if debug:
                        rb = min(BLK, NP - b * BLK)
                        nc.sync.dma_start(t_dh2[b * BLK:b * BLK + rb, :],
                                          h2[0:rb, :])
                    nc.tensor.matmul(qreg[:], lhsT=h2[:],
                                     rhs=cps[:, b * G:(b + 1) * G],
                                     start=(b == 0), stop=(b == NBLK - 1))

                if mode == "indirect":
                    T2 = l2["T2"]
                    toff2 = l2["toff2"]
                    T2MAX = int(T2.max())
                    for g in range(NGRP):
                        blocks = list(range(g * BPG, min((g + 1) * BPG, NBLK)))
                        bank = psagg.tile([BLK, 512], dt.float32, tag="agg",
                                          name=f"ab{g}")
                        for kb, b in enumerate(blocks):
                            T = int(T2[b])
                            t0 = int(toff2[b])
                            gt = g2p.tile([BLK, T2MAX, H], dt.float16, tag="g",
                                          name=f"g2_{b}")
                            nc.gpsimd.indirect_dma_start(
                                gt[:, 0:T, :], None,
                                t_msrc[:, :],
                                IndirectOffsetOnAxis(ap=offs[:, t0:t0 + T], axis=0))
                            st = seg2p.tile([BLK, T2MAX * BLK], dt.float8e4,
                                            tag="seg", name=f"s2_{b}")
                            nc.scalar.dma_start(st[:, 0:T * BLK],
                                                t_seg[:, t0 * BLK:(t0 + T) * BLK])
                            reg = bank[:, kb * H:(kb + 1) * H]
                            for t in range(T):
                                nc.tensor.matmul(
                                    reg,
                                    lhsT=st[:, t * BLK:(t + 1) * BLK],
                                    rhs=gt[:, t, :],
                                    start=(t == 0), stop=(t == T - 1))
                            epilogue(b, reg)
                else:
                    T2c = l2["T2c"]
                    Vbc = l2["Vbc"]
                    tile_off = l2["tile_off"]
                    gc_base = l2["gc_base"]
                    gc_ntiles = l2["gc_ntiles"]
                    TGC_MAX = l2["TGC_MAX"]
                    T2C_MAX = int(T2c.max())
                    for w in range(2):
                        # publish this wave's two src-quarter regions; the
                        # first pair only waits on half of phase 1
                        for q in (2 * w, 2 * w + 1):
                            nc.gpsimd.collective_compute(
                                "AllGather", mybir.AluOpType.bypass,
                                replica_groups=[list(range(P))],
                                ins=[t_min[QROW[q]:QROW[q + 1], :]],
                                outs=[t_mfull[QBASE[q]:QBASE[q + 1], :]])
                        for g in range(NGRP):
                            blocks = list(range(g * BPG, min((g + 1) * BPG, NBLK)))
                            its = {}
                            sts = {}
                            for c in (2 * w, 2 * w + 1):
                                if w == 0 and (g, c) in head_it:
                                    its[c] = head_it[(g, c)]
                                    sts[c] = head_st[(g, c)]
                                    continue
                                ntile = int(gc_ntiles[g, c])
                                nidx = ntile * BLK
                                base = int(gc_base[g, c])
                                it = idx2p.tile([BLK, TGC_MAX * 8], dt.int16,
                                                tag="idx", name=f"i2_{g}_{c}")
                                nc.sync.dma_start(
                                    it[:, 0:nidx // 16],
                                    t_off[:, base * 8:base * 8 + nidx // 16])
                                st = seg2p.tile([BLK, TGC_MAX * BLK], dt.float8e4,
                                                tag="seg", name=f"s2_{g}_{c}")
                                nc.scalar.dma_start(
                                    st[:, 0:nidx],
                                    t_seg[:, base * BLK:base * BLK + nidx])
                                its[c] = it
                                sts[c] = st
                            bank = psagg.tile([BLK, 512], dt.float32, tag="agg",
                                              name=f"ab{w}_{g}")
                            for kb, b in enumerate(blocks):
                                gts = {}
                                for c in (2 * w, 2 * w + 1):
                                    T = int(T2c[b, c])
                                    V = int(Vbc[b, c])
                                    tloc = int(tile_off[b, c] - gc_base[g, c])
                                    gt = g2p.tile([BLK, T2C_MAX, H], dt.float16,
                                                  tag="g", name=f"g{b}_{c}")
                                    nc.gpsimd.dma_gather(
                                        gt[:, 0:T, :],
                                        t_msrc[QBASE[c]:QBASE[c + 1], :],
                                        its[c][:, tloc * 8:(tloc + T) * 8],
                                        T * BLK, V, H,
                                        single_packet=True, queue_num=nextq())
                                    gts[c] = gt
                                reg = bank[:, kb * H:(kb + 1) * H]
                                nmm = sum(int(T2c[b, c]) for c in (2 * w, 2 * w + 1))
                                k = 0
                                for c in (2 * w, 2 * w + 1):
                                    T = int(T2c[b, c])
                                    V = int(Vbc[b, c])
                                    t0 = int(tile_off[b, c] - gc_base[g, c])
                                    for t in range(T):
                                        # last tile only has V-(T-1)*128
                                        # valid slots
                                        K = min(BLK, V - t * BLK)
                                        nc.tensor.matmul(
                                            reg,
                                            lhsT=sts[c][0:K, (t0 + t) * BLK:(t0 + t + 1) * BLK],
                                            rhs=gts[c][0:K, t, :],
                                            start=(k == 0), stop=(k == nmm - 1))
                                        k += 1
                                if w == 0:
                                    nc.vector.tensor_copy(
                                        acc[:, b * H:(b + 1) * H], reg)
                                else:
                                    epilogue(b, reg)

                # ---- AllReduce pooled partials (pre-W3 fold) ----
                qsb = blk2p.tile([BLK, G], dt.float32, tag="qsb", name="qsb")
                nc.scalar.copy(qsb[:], qreg[:])
                nc.sync.dma_start(t_arin[:], qsb[:])
                nc.gpsimd.collective_compute(
                    "AllReduce", mybir.AluOpType.add,
                    replica_groups=[list(range(P))],
                    ins=[t_arin[:]], outs=[t_arout[:]])

            # ---- head: pooled = W3^T q + b3; out = Wlin^T pooled + blin ----
            with tc.tile_pool(name="tail", bufs=1) as tp, \
                 tc.tile_pool(name="ps_o", bufs=1, space="PSUM") as pso:
                qt = tp.tile([BLK, G], dt.float32)
                nc.sync.dma_start(qt[:], t_arout[:])
                pw3 = pso.tile([BLK, G], dt.float32)
                nc.tensor.matmul(pw3[:], lhsT=w3s[:], rhs=qt[:],
                                 start=True, stop=True)
                pooled = tp.tile([BLK, G], dt.float32)
                nc.scalar.activation(pooled[:], pw3[:], AF.Identity,
                                     bias=b3s[:, 0:1])
                po = pso.tile([C, G], dt.float32)
                nc.tensor.matmul(po[:], lhsT=wls[:], rhs=pooled[:],
                                 start=True, stop=True)
                osb = tp.tile([C, G], dt.float32)
                nc.scalar.activation(osb[:], po[:], AF.Identity, bias=bls[:, 0:1])
                nc.sync.dma_start(t_out[:], osb[:])

    nc.compile()
    return nc


def kernel(**inputs):
    from concourse.bass_utils import run_bass_kernel_spmd

    x = np.asarray(inputs["x"], np.float32)
    edge_index = np.asarray(inputs["edge_index"], np.int64)
    batch = np.asarray(inputs["batch"], np.int64)
    W1 = np.asarray(inputs["W1"], np.float32)
    b1 = np.asarray(inputs["b1"], np.float32)
    W2 = np.asarray(inputs["W2"], np.float32)
    b2 = np.asarray(inputs["b2"], np.float32)
    W3 = np.asarray(inputs["W3"], np.float32)
    b3 = np.asarray(inputs["b3"], np.float32)
    Wlin = np.asarray(inputs["Wlin"], np.float32)
    blin = np.asarray(inputs["blin"], np.float32)

    meta, per_core = _host_prep(x, edge_index, batch)

    key = "nc"
    if key not in _CACHE:
        _CACHE[key] = _build(meta)
    nc = _CACHE[key]

    in_maps = []
    for i in range(P):
        pc = per_core[i]
        in_maps.append({
            "xe": pc["xe"], "offs": pc["offs"], "seg": pc["seg"],
            "dso": pc["dso"], "cp": pc["cp"],
            "w1": W1.astype(np.float32), "w2": W2, "w3": W3, "wl": Wlin,
            "b1b": b1.reshape(BLK, 1).astype(np.float32),
            "b2b": np.tile(b2, (BLK, 1)).astype(np.float32),
            "b3c": b3.reshape(BLK, 1).astype(np.float32),
            "blc": blin.reshape(C, 1).astype(np.float32),
            "ident": np.eye(BLK, dtype=np.float32),
        })

    trace = os.environ.get("GCN_TRACE", "0") == "1"
    res = run_bass_kernel_spmd(nc, in_maps, core_ids=list(range(P)), trace=trace)
    if trace:
        print("HW exec time:", res.exec_time_ns, "ns")
        if res.instructions_and_trace:
            print("trace:", res.instructions_and_trace[1])
    if os.environ.get("GCN_DEBUG", "0") == "1":
        kernel.debug_results = res.results
    out = res.results[0]["out"]  # [4, 64]
    return np.ascontiguousarray(out.T).astype(np.float32)
